# revision 1
# baseline (speedup 1.0000x reference)
"""BayesianLinear Trainium2 kernel, 8-core SPMD (data-parallel over batch).

Per-core computation (4 samples each):
    w_b = weight_mean + noise_b * exp(0.5 * weight_logvar)   (B,O,I)
    out_b = x_b @ w_b^T + bias                               (B,L,O)

Design (per core):
  - std = exp(0.5*logvar) once on ACT, kept natural (O on partitions).
  - mean^T resident (PE-transposed once at startup).
  - Per sample, software-pipelined in column halves so PE stays dense:
      [x load + PE-transpose x (ACT rounds to fp32r on evac)]
      [noise chunks 0,1: DVE/GpSimd scale-mul (fp32r round), PE transpose,
       DVE fused mean-add evac -> w^T cols 0-511]
      [matmuls n=0: psum += x^T.T @ w^T over 8 k-tiles, K=1 bias matmul,
       ACT evac, store]
      [chunks 2,3 -> w^T cols 512-1023]  [matmuls n=1]
  - fp32r matmuls run the PE at bf16 rate (1 cyc/row); fp32r transposes are
    exact permutations; rounding (~2^-12) happens once per operand.
"""
import numpy as np

SAMPLES = 4           # batch samples per core
N_CORES = 8
B, L, I, O = 32, 512, 1024, 1024
KT = I // 128         # 8 k-tiles (contraction)
OT = O // 128         # 8 o-blocks
LT = L // 128         # 4 l-tiles
NCH = 4               # noise chunks per sample (2 o-blocks each)

_cache = {}


def _split_multi_waits(nc, mybir):
    """This walrus build allows at most one sync-wait per instruction; move
    extra waits onto preceding single-wait NOPs on the same engine.  Safe
    because kernel semaphores are monotonic between resets, so waiting
    sequentially is equivalent to waiting on the conjunction."""
    for fn in nc.m.functions:
        for bb in fn.blocks:
            insts = bb.instructions
            changed = False
            new_list = []
            for inst in insts:
                si = inst.sync_info
                if si is not None and si.on_wait and len(si.on_wait) > 1:
                    waits = list(si.on_wait)
                    for j, w in enumerate(waits[:-1]):
                        nop = mybir.InstNoOp(name=f"{inst.name}-w{j}", ins=[], outs=[])
                        nop.engine = inst.engine
                        nop.sync_info = mybir.SyncInfo(on_wait=[w], on_update=[])
                        new_list.append(nop)
                    inst.sync_info = mybir.SyncInfo(
                        on_wait=[waits[-1]], on_update=list(si.on_update or []))
                    changed = True
                new_list.append(inst)
            if changed:
                bb.instructions = new_list


def build_nc(use_f32r=True):
    from contextlib import ExitStack
    from concourse import bass, mybir, tile, masks

    F32 = mybir.dt.float32
    F32R = mybir.dt.float32r if use_f32r else mybir.dt.float32
    Exp = mybir.ActivationFunctionType.Exp
    Copy = mybir.ActivationFunctionType.Copy
    mult = mybir.AluOpType.mult
    add = mybir.AluOpType.add

    nc = bass.Bass()
    x_d = nc.declare_dram_parameter("x", [SAMPLES, L, I], F32, isOutput=False)
    nz_d = nc.declare_dram_parameter("noise", [SAMPLES, O, I], F32, isOutput=False)
    wm_d = nc.declare_dram_parameter("weight_mean", [O, I], F32, isOutput=False)
    wl_d = nc.declare_dram_parameter("weight_logvar", [O, I], F32, isOutput=False)
    b_d = nc.declare_dram_parameter("bias", [O], F32, isOutput=False)
    out_d = nc.declare_dram_parameter("out", [SAMPLES, L, O], F32, isOutput=True)

    with tile.TileContext(nc) as tc, ExitStack() as ctx:
        resident = ctx.enter_context(tc.tile_pool(name="resident", bufs=1))
        nat_pool = ctx.enter_context(tc.tile_pool(name="nat", bufs=3))
        scn_pool = ctx.enter_context(tc.tile_pool(name="scn", bufs=2))
        xnat_pool = ctx.enter_context(tc.tile_pool(name="xnat", bufs=2))
        wx_pool = ctx.enter_context(tc.tile_pool(name="wx", bufs=1))
        out_pool = ctx.enter_context(tc.tile_pool(name="outp", bufs=3))
        psum_mm = ctx.enter_context(tc.tile_pool(name="psum_mm", bufs=2, space="PSUM"))
        psum_nt = ctx.enter_context(tc.tile_pool(name="psum_nt", bufs=3, space="PSUM"))
        psum_xt = ctx.enter_context(tc.tile_pool(name="psum_xt", bufs=3, space="PSUM"))

        # ---------------- one-time setup ----------------
        std_nat = resident.tile([128, OT, I], F32, tag="std")    # exp(.5 lv), natural
        meanT = resident.tile([128, KT, O], F32, tag="meanT")    # mean^T
        ident = resident.tile([128, 128], F32, tag="ident")
        ident_r = resident.tile([128, 128], F32R, tag="ident_r")
        ones_f = resident.tile([1, 128], F32, tag="ones_f")
        ones_r = resident.tile([1, 128], F32R, tag="ones_r")
        bias_f = resident.tile([1, O], F32, tag="bias_f")
        bias_r = resident.tile([1, O], F32R, tag="bias_r")

        masks.make_identity(nc, ident[:])
        nc.vector.tensor_copy(ident_r[:], ident[:])
        nc.vector.memset(ones_f[:], 1.0)
        nc.vector.tensor_copy(ones_r[:], ones_f[:])
        nc.sync.dma_start(bias_f[:], b_d[:].rearrange("(a o) -> a o", a=1))
        nc.vector.tensor_copy(bias_r[:], bias_f[:])

        # PE pre-warm: short burst of dummy transposes (self as moving operand,
        # result is garbage, never read) trips the HAM activity window so the
        # 2.4 GHz clock is ungated when real work arrives.
        warm = resident.tile([128, 128], F32, tag="warm")
        nc.gpsimd.memset(warm[:], 0.001)
        pwarm = psum_xt.tile([128, 4, 128], F32, tag="pxt")
        for _ in range(16):
            nc.tensor.matmul(pwarm[:, 0, :], warm[:], warm[:],
                             is_transpose=True, start=True, stop=True)

        def emit_mean_slab(j):
            """load + exp + transpose weight slab j (o-blocks 2j, 2j+1)."""
            sl = slice(256 * j, 256 * (j + 1))
            mt = nat_pool.tile([128, 2, I], F32, tag="nat", name=f"mt{j}")
            nc.sync.dma_start(
                mt[:], wm_d[sl, :].rearrange("(q p) i -> p q i", p=128))
            nc.sync.dma_start(std_nat[:, 2 * j:2 * (j + 1), :],
                              wl_d[sl, :].rearrange("(q p) i -> p q i", p=128))
            nc.scalar.activation(std_nat[:, 2 * j:2 * (j + 1), :],
                                 std_nat[:, 2 * j:2 * (j + 1), :],
                                 Exp, bias=0.0, scale=0.5)
            for q in range(2):
                ob = 2 * j + q
                for kh in range(2):  # k halves of 4
                    px = psum_xt.tile([128, 4, 128], F32, tag="pxt")
                    for kk in range(4):
                        k = 4 * kh + kk
                        nc.tensor.matmul(
                            px[:, kk, :], mt[:, q, 128 * k:128 * (k + 1)], ident[:],
                            is_transpose=True, start=True, stop=True)
                    nc.vector.tensor_copy(
                        meanT[:, 4 * kh:4 * (kh + 1), 128 * ob:128 * (ob + 1)], px[:])

        # ---------------- per-sample pipeline ----------------
        def emit_chunk(b, c, wT):
            """noise chunk c (o-blocks 2c, 2c+1): load, scale, transpose, add."""
            nz = nat_pool.tile([128, 2, I], F32, tag="nat")
            nc.sync.dma_start(
                nz[:], nz_d[b, 256 * c:256 * (c + 1), :].rearrange("(q p) i -> p q i", p=128))
            sc = scn_pool.tile([128, 2, I], F32R, tag="scn")
            # alternate scale-muls between DVE and GpSimd
            eng = nc.gpsimd if c == 3 else nc.vector
            eng.tensor_tensor(sc[:], nz[:], std_nat[:, 2 * c:2 * (c + 1), :], mult)
            for q in range(2):
                ob = 2 * c + q
                for kh in range(2):
                    pn = psum_nt.tile([128, 4, 128], F32R, tag="pnt")
                    for kk in range(4):
                        k = 4 * kh + kk
                        nc.tensor.matmul(
                            pn[:, kk, :], sc[:, q, 128 * k:128 * (k + 1)],
                            ident_r[:], is_transpose=True, start=True, stop=True)
                    nc.vector.tensor_tensor(
                        wT[:, 4 * kh:4 * (kh + 1), 128 * ob:128 * (ob + 1)],
                        pn[:], meanT[:, 4 * kh:4 * (kh + 1), 128 * ob:128 * (ob + 1)],
                        add)

        def emit_mm_half(b, n, wT, xT):
            """matmuls for output columns [512n, 512(n+1))."""
            for m in range(LT):
                pm = psum_mm.tile([128, 512], F32, tag="pmm")
                for k in range(KT):
                    nc.tensor.matmul(pm[:], xT[:, k, 128 * m:128 * (m + 1)],
                                     wT[:, k, 512 * n:512 * (n + 1)],
                                     start=(k == 0), stop=False)
                nc.tensor.matmul(pm[:], ones_r[:], bias_r[:, 512 * n:512 * (n + 1)],
                                 start=False, stop=True)
                ot = out_pool.tile([128, 512], F32, tag="out")
                nc.scalar.activation(ot[:], pm[:], Copy)
                nc.scalar.dma_start(
                    out_d[b, 128 * m:128 * (m + 1), 512 * n:512 * (n + 1)], ot[:])

        x_tiles = {0: xnat_pool.tile([128, LT, I], F32, tag="xnat", name="xn0")}
        nc.sync.dma_start(x_tiles[0][:], x_d[0].rearrange("(m p) i -> p m i", p=128))
        for b in range(SAMPLES):
            # x^T build (ACT rounds on evac)
            xT = wx_pool.tile([128, KT, L], F32R, tag="xT")
            x_nat = x_tiles.pop(b)
            for m in range(LT):
                for kh in range(2):
                    px = psum_xt.tile([128, 4, 128], F32, tag="pxt")
                    for kk in range(4):
                        k = 4 * kh + kk
                        nc.tensor.matmul(
                            px[:, kk, :], x_nat[:, m, 128 * k:128 * (k + 1)], ident[:],
                            is_transpose=True, start=True, stop=True)
                    nc.scalar.activation(
                        xT[:, 4 * kh:4 * (kh + 1), 128 * m:128 * (m + 1)], px[:], Copy)
            if b + 1 < SAMPLES:
                xn_next = xnat_pool.tile([128, LT, I], F32, tag="xnat", name=f"xn{b+1}")
                x_tiles[b + 1] = xn_next
                nc.sync.dma_start(
                    x_tiles[b + 1][:], x_d[b + 1].rearrange("(m p) i -> p m i", p=128))
            wT = wx_pool.tile([128, KT, O], F32R, tag="wT")
            for half in range(2):
                for cc in (2 * half, 2 * half + 1):
                    if b == 0:
                        emit_mean_slab(cc)
                    emit_chunk(b, cc, wT)
                emit_mm_half(b, half, wT, xT)

    _split_multi_waits(nc, mybir)
    return nc


def _get_nc(use_f32r=True):
    key = ("nc", use_f32r)
    if key not in _cache:
        _cache[key] = build_nc(use_f32r)
    return _cache[key]


def kernel(x, weight_mean, weight_logvar, bias, noise):
    from concourse import bass_utils

    x = np.ascontiguousarray(x, dtype=np.float32)
    noise = np.ascontiguousarray(noise, dtype=np.float32)
    weight_mean = np.ascontiguousarray(weight_mean, dtype=np.float32)
    weight_logvar = np.ascontiguousarray(weight_logvar, dtype=np.float32)
    bias = np.ascontiguousarray(bias, dtype=np.float32)

    nc = _get_nc()
    in_maps = []
    for c in range(N_CORES):
        sl = slice(SAMPLES * c, SAMPLES * (c + 1))
        in_maps.append({
            "x": x[sl], "noise": noise[sl],
            "weight_mean": weight_mean, "weight_logvar": weight_logvar,
            "bias": bias,
        })
    res = bass_utils.run_bass_kernel_spmd(nc, in_maps, list(range(N_CORES)))
    out = np.concatenate([res.results[c]["out"] for c in range(N_CORES)], axis=0)
    return out.astype(np.float32)



# revision 3
# speedup vs baseline: 1.0811x; 1.0811x over previous
"""BayesianLinear Trainium2 kernel, 8-core SPMD (data-parallel over batch).

Per-core computation (4 samples each):
    w_b = weight_mean + noise_b * exp(0.5 * weight_logvar)   (B,O,I)
    out_b = x_b @ w_b^T + bias                               (B,L,O)

Design (per core), v2 — bf16 matmul datapath:
  - std = exp(.5 lv) once on ACT (cast to bf16 on write), O on partitions.
  - mean^T resident in bf16 (PE-transposed once at startup, cast on evac).
  - x loaded row-block-major ("(p m) i"): 16KB contiguous per partition
    descriptor; the induced L-permutation is undone for free by the
    partition->DRAM-row map of the output store.
  - Per sample, software-pipelined in column halves so PE stays dense:
      noise chunk: load f32 -> DVE/GpSimd scale-mul (bf16 out) ->
      PE transpose (bf16, 1 cyc/row) -> DVE fused mean-add evac (bf16 wT)
      matmuls: psum += xT.T @ wT over 8 k-tiles (bf16, 1 cyc/row),
      K=1 bias matmul, ACT evac (f32) into a full-width out slab,
      single 512KB store per (sample, row-block) with 4KB-contiguous rows.
"""
import numpy as np

SAMPLES = 4           # batch samples per core
N_CORES = 8
B, L, I, O = 32, 512, 1024, 1024
KT = I // 128         # 8 k-tiles (contraction)
OT = O // 128         # 8 o-blocks
LT = L // 128         # 4 l-tiles (row blocks)
NCH = 4               # noise chunks per sample (2 o-blocks each)

_cache = {}


def _split_multi_waits(nc, mybir):
    """This walrus build allows at most one sync-wait per instruction; move
    extra waits onto preceding single-wait NOPs on the same engine.  Safe
    because kernel semaphores are monotonic between resets, so waiting
    sequentially is equivalent to waiting on the conjunction."""
    for fn in nc.m.functions:
        for bb in fn.blocks:
            insts = bb.instructions
            changed = False
            new_list = []
            for inst in insts:
                si = inst.sync_info
                if si is not None and si.on_wait and len(si.on_wait) > 1:
                    waits = list(si.on_wait)
                    for j, w in enumerate(waits[:-1]):
                        nop = mybir.InstNoOp(name=f"{inst.name}-w{j}", ins=[], outs=[])
                        nop.engine = inst.engine
                        nop.sync_info = mybir.SyncInfo(on_wait=[w], on_update=[])
                        new_list.append(nop)
                    inst.sync_info = mybir.SyncInfo(
                        on_wait=[waits[-1]], on_update=list(si.on_update or []))
                    changed = True
                new_list.append(inst)
            if changed:
                bb.instructions = new_list


def build_nc(use_f32r=True):
    from contextlib import ExitStack
    from concourse import bass, mybir, tile, masks

    F32 = mybir.dt.float32
    BF16 = mybir.dt.bfloat16
    Exp = mybir.ActivationFunctionType.Exp
    Copy = mybir.ActivationFunctionType.Copy
    mult = mybir.AluOpType.mult
    add = mybir.AluOpType.add

    nc = bass.Bass()
    x_d = nc.declare_dram_parameter("x", [SAMPLES, L, I], F32, isOutput=False)
    nz_d = nc.declare_dram_parameter("noise", [SAMPLES, O, I], F32, isOutput=False)
    wm_d = nc.declare_dram_parameter("weight_mean", [O, I], F32, isOutput=False)
    wl_d = nc.declare_dram_parameter("weight_logvar", [O, I], F32, isOutput=False)
    b_d = nc.declare_dram_parameter("bias", [O], F32, isOutput=False)
    out_d = nc.declare_dram_parameter("out", [SAMPLES, L, O], F32, isOutput=True)

    with tile.TileContext(nc) as tc, ExitStack() as ctx:
        resident = ctx.enter_context(tc.tile_pool(name="resident", bufs=1))
        nz_pool = ctx.enter_context(tc.tile_pool(name="nz", bufs=4))
        sc_pool = ctx.enter_context(tc.tile_pool(name="scn", bufs=2))
        xnat_pool = ctx.enter_context(tc.tile_pool(name="xnat", bufs=2))
        xT_pool = ctx.enter_context(tc.tile_pool(name="xT", bufs=2))
        wT_pool = ctx.enter_context(tc.tile_pool(name="wT", bufs=2))
        out_pool = ctx.enter_context(tc.tile_pool(name="outp", bufs=2))
        psum_mm = ctx.enter_context(tc.tile_pool(name="psum_mm", bufs=2, space="PSUM"))
        psum_t = ctx.enter_context(tc.tile_pool(name="psum_t", bufs=3, space="PSUM"))

        # ---------------- one-time setup ----------------
        std_nat = resident.tile([128, OT, I], BF16, tag="std")   # exp(.5 lv), natural
        meanT = resident.tile([128, KT, O], BF16, tag="meanT")   # mean^T
        ident_f = resident.tile([128, 128], F32, tag="ident_f")
        ident_b = resident.tile([128, 128], BF16, tag="ident_b")
        ones_b = resident.tile([1, 128], BF16, tag="ones_b")
        bias_f = resident.tile([1, O], F32, tag="bias_f")
        bias_b = resident.tile([1, O], BF16, tag="bias_b")

        masks.make_identity(nc, ident_f[:])
        nc.vector.tensor_copy(ident_b[:], ident_f[:])
        nc.gpsimd.memset(ones_b[:], 1.0)
        nc.sync.dma_start(bias_f[:], b_d[:].rearrange("(a o) -> a o", a=1))
        nc.vector.tensor_copy(bias_b[:], bias_f[:])

        # PE pre-warm: short burst of dummy transposes trips the HAM activity
        # window so the 2.4 GHz clock is ungated when real work arrives.
        warm = resident.tile([128, 128], F32, tag="warm")
        nc.gpsimd.memset(warm[:], 0.001)
        pwarm = psum_t.tile([128, 4, 128], F32, tag="pt")
        for _ in range(16):
            nc.tensor.matmul(pwarm[:, 0, :], warm[:], warm[:],
                             is_transpose=True, start=True, stop=True)

        def emit_mean_slab(j):
            """load + exp + transpose weight slab j (o-blocks 2j, 2j+1)."""
            sl = slice(256 * j, 256 * (j + 1))
            mt = nz_pool.tile([128, 2, I], F32, tag="nz", name=f"mt{j}")
            lt = nz_pool.tile([128, 2, I], F32, tag="nz", name=f"lt{j}")
            nc.sync.dma_start(
                mt[:], wm_d[sl, :].rearrange("(q p) i -> p q i", p=128))
            nc.sync.dma_start(
                lt[:], wl_d[sl, :].rearrange("(q p) i -> p q i", p=128))
            nc.scalar.activation(std_nat[:, 2 * j:2 * (j + 1), :], lt[:],
                                 Exp, bias=0.0, scale=0.5)
            for q in range(2):
                ob = 2 * j + q
                for kh in range(2):  # k halves of 4
                    px = psum_t.tile([128, 4, 128], F32, tag="pt")
                    for kk in range(4):
                        k = 4 * kh + kk
                        nc.tensor.matmul(
                            px[:, kk, :], mt[:, q, 128 * k:128 * (k + 1)], ident_f[:],
                            is_transpose=True, start=True, stop=True)
                    dst = meanT[:, 4 * kh:4 * (kh + 1), 128 * ob:128 * (ob + 1)]
                    if q == 0:
                        nc.scalar.activation(dst, px[:], Copy)
                    else:
                        nc.vector.tensor_copy(dst, px[:])

        # ---------------- per-sample pipeline ----------------
        def emit_chunk(b, c, wT):
            """noise chunk c (o-blocks 2c, 2c+1): load, scale (bf16),
            transpose, fused mean-add into wT."""
            nz = nz_pool.tile([128, 2, I], F32, tag="nz")
            nc.sync.dma_start(
                nz[:], nz_d[b, 256 * c:256 * (c + 1), :].rearrange("(q p) i -> p q i", p=128))
            sc = sc_pool.tile([128, 2, I], BF16, tag="scn")
            eng = nc.gpsimd if c == 3 else nc.vector
            eng.tensor_tensor(sc[:], nz[:], std_nat[:, 2 * c:2 * (c + 1), :], mult)
            for q in range(2):
                ob = 2 * c + q
                for kh in range(2):
                    pn = psum_t.tile([128, 4, 128], BF16, tag="pt")
                    for kk in range(4):
                        k = 4 * kh + kk
                        nc.tensor.matmul(
                            pn[:, kk, :], sc[:, q, 128 * k:128 * (k + 1)],
                            ident_b[:], is_transpose=True, start=True, stop=True)
                    nc.vector.tensor_tensor(
                        wT[:, 4 * kh:4 * (kh + 1), 128 * ob:128 * (ob + 1)],
                        pn[:], meanT[:, 4 * kh:4 * (kh + 1), 128 * ob:128 * (ob + 1)],
                        add)

        def emit_xT(xT, x_nat):
            """PE-transpose x (f32) with cast-to-bf16 on ACT evac."""
            for m in range(LT):
                for kh in range(2):
                    px = psum_t.tile([128, 4, 128], F32, tag="pt")
                    for kk in range(4):
                        k = 4 * kh + kk
                        nc.tensor.matmul(
                            px[:, kk, :], x_nat[:, m, 128 * k:128 * (k + 1)], ident_f[:],
                            is_transpose=True, start=True, stop=True)
                    nc.scalar.activation(
                        xT[:, 4 * kh:4 * (kh + 1), 128 * m:128 * (m + 1)], px[:], Copy)

        def emit_mm_half(b, n, wT, xT, osb):
            """matmuls for output columns [512n, 512(n+1)); store full rows
            after the second half."""
            for m in range(LT):
                pm = psum_mm.tile([128, 512], F32, tag="pmm")
                for k in range(KT):
                    nc.tensor.matmul(pm[:], xT[:, k, 128 * m:128 * (m + 1)],
                                     wT[:, k, 512 * n:512 * (n + 1)],
                                     start=(k == 0), stop=False)
                nc.tensor.matmul(pm[:], ones_b[:], bias_b[:, 512 * n:512 * (n + 1)],
                                 start=False, stop=True)
                nc.scalar.activation(osb[:, m, 512 * n:512 * (n + 1)], pm[:], Copy)
                if n == 1:
                    nc.scalar.dma_start(
                        out_d[b].rearrange("(p m) o -> p m o", m=LT)[:, m, :],
                        osb[:, m, :])

        x_tiles = {0: xnat_pool.tile([128, LT, I], F32, tag="xnat", name="xn0")}
        nc.sync.dma_start(x_tiles[0][:],
                          x_d[0].rearrange("(p m) i -> p m i", m=LT))
        for b in range(SAMPLES):
            xT = xT_pool.tile([128, KT, L], BF16, tag="xT")
            emit_xT(xT, x_tiles.pop(b))
            wT = wT_pool.tile([128, KT, O], BF16, tag="wT")
            osb = out_pool.tile([128, LT, O], F32, tag="out")
            for half in range(2):
                for cc in (2 * half, 2 * half + 1):
                    if b == 0:
                        emit_mean_slab(cc)
                    emit_chunk(b, cc, wT)
                    if b + 1 < SAMPLES and half == 0 and cc == 1:
                        xn_next = xnat_pool.tile([128, LT, I], F32, tag="xnat",
                                                 name=f"xn{b+1}")
                        x_tiles[b + 1] = xn_next
                        nc.sync.dma_start(
                            xn_next[:], x_d[b + 1].rearrange("(p m) i -> p m i", m=LT))
                emit_mm_half(b, half, wT, xT, osb)

    _split_multi_waits(nc, mybir)
    return nc


def _get_nc(use_f32r=True):
    key = ("nc", use_f32r)
    if key not in _cache:
        _cache[key] = build_nc(use_f32r)
    return _cache[key]


def kernel(x, weight_mean, weight_logvar, bias, noise):
    from concourse import bass_utils

    x = np.ascontiguousarray(x, dtype=np.float32)
    noise = np.ascontiguousarray(noise, dtype=np.float32)
    weight_mean = np.ascontiguousarray(weight_mean, dtype=np.float32)
    weight_logvar = np.ascontiguousarray(weight_logvar, dtype=np.float32)
    bias = np.ascontiguousarray(bias, dtype=np.float32)

    nc = _get_nc()
    in_maps = []
    for c in range(N_CORES):
        sl = slice(SAMPLES * c, SAMPLES * (c + 1))
        in_maps.append({
            "x": x[sl], "noise": noise[sl],
            "weight_mean": weight_mean, "weight_logvar": weight_logvar,
            "bias": bias,
        })
    res = bass_utils.run_bass_kernel_spmd(nc, in_maps, list(range(N_CORES)))
    out = np.concatenate([res.results[c]["out"] for c in range(N_CORES)], axis=0)
    return out.astype(np.float32)
